# Initial kernel scaffold
#
"""TRN2 Bass kernel for nn_BottleneckA (gated bottleneck MLP over 1x1 convs).

Computation (reference):
    h1 = relu(g * (W1 @ x + b1))    g = relu(gate)   per (batch, mid-channel)
    h2 = relu(g * (W2 @ h1 + b2))
    y  = relu(W3 @ h2 + b3) + x

All three matmuls run in fp8e4m3 with perf_mode=DoubleRow (2 fp8 weights
per PE cell -> 256-deep contraction per pass, ~2x bf16 FLOP rate).
Weights are pre-scaled by powers of two (S1=32, S2=16, S3=16) and the
activations stored scaled (h1 x4, h2 x2) to sit in e4m3's normal range;
every scale folds into the per-(batch,channel) ACT gate scale/bias, so
no extra device ops. The device returns t = 32*relu(W3 h2 + b3) in fp8;
the host applies y = x + t/32 with the exact fp32 x it already holds
(final rel-rms error ~7.8e-3, dominated by fp8 quantization diluted by
the residual: relu3 rms is only ~0.087 of y rms).

Sharding: data-parallel over batch B=16 across 8 NeuronCores (2/core),
weights replicated. Per core, each batch's [C=1024, HW=3136] activation
is processed in 7 spatial chunks of 448 columns (one PSUM bank per
matmul output). x/y are chunk-tiled on the host so chunk DMAs move 3584
contiguous bytes per partition (sub-512B runs run at half DMA rate).

Spatial chunks are processed in PAIRS sharing weight loads: conv2/conv3
emit LDW, MM(chunkA), MM(chunkB) (singleton accumulation groups stay
adjacent through the Tile scheduler) and a post-legalization pass
deletes the duplicate back-to-back LDWs. LDWEIGHTS only partially
overlaps DoubleRow matmuls on this hardware (~90ns effective per LDW on
top of the 211ns MM), so fewer LDWs means real PE time. conv1's rigid
4-MM accumulation chains are scheduled contiguously per chunk and cannot
share LDWs. conv3's per-chunk output posts (relu + b3, PSUM->fp8) are
split 12-DVE / 4-ACT per pair, balancing both engines under the PE pace;
stores ride the otherwise-idle GPSIMD SWDGE queue so no compute queue
ever blocks on a store's data dependency.
"""
import os
import time

import numpy as np

import concourse.bass as bass
import concourse.tile as tile
from concourse import mybir, bass2jax
from concourse.bass2jax import _bass_exec_p, install_neuronx_cc_hook
from contextlib import ExitStack

import jax
from jax.sharding import Mesh, PartitionSpec
from jax.experimental.shard_map import shard_map

B, C, MID, HW = 16, 1024, 256, 56 * 56
NCORES = 8
BPC = B // NCORES
NC_CHUNK = 448
NCHUNKS = HW // NC_CHUNK
KO = C // 128
M2 = MID // 128
f32 = mybir.dt.float32
f8 = mybir.dt.float8e4

S1, S2, S3 = 32.0, 16.0, 16.0
SH1, SH2 = 4.0, 2.0
OUT_SCALE = S3 * SH2

_EVS_CAP = 2


def _split_excess_waits(nc):
    """This container's walrus accepts only 1 sync-wait slot on most ISA
    structs while Tile emits 2-3; hoist the excess onto preceding
    InstEventSemaphore ops on the same (FIFO) engine queue."""
    counter = [0]
    for fn in nc.m.functions:
        for blk in fn.blocks:
            new_insts = []
            for inst in blk.instructions:
                si = inst.sync_info
                waits = list(si.on_wait) if si is not None else []
                cap = _EVS_CAP if type(inst).__name__ == "InstEventSemaphore" else 1
                if len(waits) > cap:
                    excess, keep = waits[: len(waits) - cap], waits[len(waits) - cap:]
                    while excess:
                        chunk, excess = excess[:_EVS_CAP], excess[_EVS_CAP:]
                        counter[0] += 1
                        new_insts.append(mybir.InstEventSemaphore(
                            name=f"EVSW-{counter[0]}-{inst.name}",
                            engine=inst.engine,
                            ins=[], outs=[],
                            sync_info=mybir.SyncInfo(on_wait=list(chunk), on_update=[]),
                        ))
                    inst.sync_info = mybir.SyncInfo(
                        on_wait=keep, on_update=list(si.on_update))
                new_insts.append(inst)
            blk.instructions = new_insts


def _ldw_sig(inst):
    a = inst.ins[0]
    return (a.memref, a.offset, tuple(tuple(p) for p in a.ap), str(a.dtype),
            str(inst.perf_mode))


def _dedup_ldweights(nc):
    """Remove an InstLdweights whose weights AP matches the immediately
    preceding LDW on the PE stream with only Matmults in between (the
    weights are already loaded). Waits on the removed LDW migrate to the
    next instruction (hoisted later by _split_excess_waits)."""
    removed = 0
    PE = mybir.EngineType.PE
    for fn in nc.m.functions:
        for blk in fn.blocks:
            new = []
            last_sig = None
            pending_waits = []
            for inst in blk.instructions:
                tn = type(inst).__name__
                if getattr(inst, "engine", None) == PE:
                    if tn == "InstLdweights":
                        sig = _ldw_sig(inst)
                        if sig == last_sig:
                            si = inst.sync_info
                            if si is not None:
                                assert not si.on_update, "LDW with updates"
                                pending_waits.extend(si.on_wait)
                            removed += 1
                            continue
                        last_sig = sig
                    elif tn not in ("InstMatmult", "InstEventSemaphore"):
                        # control flow / drains: be conservative
                        last_sig = None
                    if pending_waits and tn != "InstLdweights":
                        si = inst.sync_info
                        waits = list(si.on_wait) if si is not None else []
                        upd = list(si.on_update) if si is not None else []
                        inst.sync_info = mybir.SyncInfo(
                            on_wait=pending_waits + waits, on_update=upd)
                        pending_waits = []
                new.append(inst)
            # leftover waits at block end: attach to nothing is unsafe
            assert not pending_waits
            blk.instructions = new
    return removed


def build_bass(repeat: int = 1) -> bass.Bass:
    nc = bass.Bass(trn_type="TRN2")
    xs = nc.dram_tensor("xs", [BPC, NCHUNKS, 128, KO * NC_CHUNK], f8,
                        kind="ExternalInput")
    w1t = nc.dram_tensor("w1t", [128, KO, MID], f8, kind="ExternalInput")
    w2t = nc.dram_tensor("w2t", [128, M2, MID], f8, kind="ExternalInput")
    w3t = nc.dram_tensor("w3t", [128, M2, C], f8, kind="ExternalInput")
    vecs = nc.dram_tensor("vecs", [128, BPC, 8], f32, kind="ExternalInput")
    b3v = nc.dram_tensor("b3v", [128, KO], f32, kind="ExternalInput")
    ys = nc.dram_tensor("ys", [BPC, NCHUNKS, 128, KO * NC_CHUNK], f8,
                        kind="ExternalOutput")

    Relu = mybir.ActivationFunctionType.Relu
    DR = mybir.MatmulPerfMode.DoubleRow

    with tile.TileContext(nc) as tc, ExitStack() as ctx:
        wpool = ctx.enter_context(tc.tile_pool(name="w", bufs=1))
        xpool = ctx.enter_context(tc.tile_pool(name="x", bufs=8))
        hpool = ctx.enter_context(tc.tile_pool(name="h", bufs=4))
        opool = ctx.enter_context(tc.tile_pool(name="o", bufs=4))
        pp1 = ctx.enter_context(tc.tile_pool(name="pp1", bufs=3, space="PSUM"))
        pp2 = ctx.enter_context(tc.tile_pool(name="pp2", bufs=2, space="PSUM"))
        pp3 = ctx.enter_context(tc.tile_pool(name="pp3", bufs=3, space="PSUM"))

        w1_sb = wpool.tile([128, KO, MID], f8, tag="w1")
        nc.scalar.dma_start(w1_sb[:], w1t[:])
        v_sb = wpool.tile([128, BPC, 8], f32, tag="v")
        nc.scalar.dma_start(v_sb[:], vecs[:])
        w2_sb = wpool.tile([128, M2, MID], f8, tag="w2")
        nc.scalar.dma_start(w2_sb[:], w2t[:])
        w3_sb = wpool.tile([128, M2, C], f8, tag="w3")
        nc.scalar.dma_start(w3_sb[:], w3t[:])
        b3_sb = wpool.tile([128, KO], f32, tag="b3")
        nc.scalar.dma_start(b3_sb[:], b3v[:])

        chunks = [(b, ci) for b in range(BPC) for ci in range(NCHUNKS)]
        n = len(chunks)
        NP = n // 2            # chunk pairs

        def emit_load(i, halves=1):
            b, ci = chunks[i]
            x_t = xpool.tile([128, KO, NC_CHUNK], f8, tag="xt")
            src = xs[b, ci].rearrange("p (ko n) -> p ko n", ko=KO)
            step = KO // halves
            for h in range(halves):
                nc.sync.dma_start(x_t[:, h * step:(h + 1) * step, :],
                                  src[:, h * step:(h + 1) * step, :])
            return x_t

        def conv1_chunk(i, xts, ps1s):
            """per-chunk conv1 (the scheduler keeps each accumulation
            chain contiguous, so chunk-pair LDW sharing is not possible
            here; chains stay per chunk)."""
            x_t = xts[i]
            ps1s[i] = []
            for m in range(M2):
                ps = pp1.tile([128, NC_CHUNK], f32, tag="ps1", name="ps1t")
                for kd in range(KO // 2):
                    nc.tensor.matmul(
                        ps[:], w1_sb[:, 2 * kd:2 * kd + 2,
                                     m * 128:(m + 1) * 128],
                        x_t[:, 2 * kd:2 * kd + 2, :],
                        start=(kd == 0), stop=(kd == KO // 2 - 1),
                        perf_mode=DR)
                ps1s[i].append(ps)

        def h1_chunk(i, ps1s, h1s):
            b, _ = chunks[i]
            h1 = hpool.tile([128, M2, NC_CHUNK], f8, tag="h1", name="h1t")
            for m in range(M2):
                nc.scalar.activation(h1[:, m, :], ps1s[i][m][:], Relu,
                                     bias=v_sb[:, b, 2 + m:3 + m],
                                     scale=v_sb[:, b, m:m + 1])
            h1s[i] = h1
            del ps1s[i]

        def c2h2_m(q, m, h1s, h2s):
            """conv2 m-tile for both chunks of pair q (one shared LDW:
            the two MMs are singleton groups and stay adjacent) + h2."""
            pss = []
            for ab in range(2):
                i = 2 * q + ab
                ps = pp2.tile([128, NC_CHUNK], f32, tag="ps2", name="ps2t")
                nc.tensor.matmul(ps[:], w2_sb[:, :, m * 128:(m + 1) * 128],
                                 h1s[i][:, :, :], start=True, stop=True,
                                 perf_mode=DR)
                pss.append(ps)
            for ab in range(2):
                i = 2 * q + ab
                b, _ = chunks[i]
                nc.scalar.activation(h2s[i][:, m, :], pss[ab][:], Relu,
                                     bias=v_sb[:, b, 6 + m:7 + m],
                                     scale=v_sb[:, b, 4 + m:5 + m])

        def alloc_h2(q, h2s):
            for ab in range(2):
                h2s[2 * q + ab] = hpool.tile([128, M2, NC_CHUNK], f8,
                                             tag="h2", name="h2t")

        def emit_pair_iter(q, xts, ps1s, h1s, h2s, last):
            """Iter q: conv3+posts+stores for pair q; conv2/h2 for pair
            q+1; conv1+h1 for pair q+2; loads for pair q+3. The PE stream
            interleaves conv3 pair-steps (shared LDW) with the rest."""
            if q + 3 < NP:
                for ab in range(2):
                    xts[2 * (q + 3) + ab] = emit_load(2 * (q + 3) + ab)
            h2A, h2B = h2s.pop(2 * q), h2s.pop(2 * q + 1)
            if q + 2 < NP:
                alloc_h2(q + 2, h2s)
            o_ts, dsts = [], []
            for ab in range(2):
                i = 2 * q + ab
                b, ci = chunks[i]
                o_ts.append(opool.tile([128, KO, NC_CHUNK], f8, tag="ot",
                                       name="ot"))
                dsts.append(ys[b, ci].rearrange("p (m n) -> p m n", m=KO))

            def c3_step(m8):
                """one shared-weight conv3 step: LDW w3[m8]; MM A; MM B."""
                pss = []
                for h2 in (h2A, h2B):
                    ps = pp3.tile([128, NC_CHUNK], f32, tag="ps3",
                                  name="ps3t")
                    nc.tensor.matmul(ps[:],
                                     w3_sb[:, :, m8 * 128:(m8 + 1) * 128],
                                     h2[:, :, :], start=True, stop=True,
                                     perf_mode=DR)
                    pss.append(ps)
                return pss

            def posts(m8, pss):
                on_act = m8 in (0, 2)
                for ab in range(2):
                    if on_act:
                        nc.scalar.activation(o_ts[ab][:, m8, :], pss[ab][:],
                                             Relu, bias=b3_sb[:, m8:m8 + 1],
                                             scale=1.0)
                    else:
                        nc.vector.tensor_scalar(o_ts[ab][:, m8, :],
                                                pss[ab][:],
                                                b3_sb[:, m8:m8 + 1], 0.0,
                                                mybir.AluOpType.add,
                                                mybir.AluOpType.max)

            posts(0, c3_step(0))
            posts(1, c3_step(1))
            if q + 1 < NP:
                c2h2_m(q + 1, 0, h1s, h2s)
            posts(2, c3_step(2))
            posts(3, c3_step(3))
            if last:
                nc.gpsimd.dma_start(dsts[0][:, :4, :], o_ts[0][:, :4, :])
                nc.gpsimd.dma_start(dsts[1][:, :4, :], o_ts[1][:, :4, :])
            if q + 1 < NP:
                c2h2_m(q + 1, 1, h1s, h2s)
            posts(4, c3_step(4))
            if q + 2 < NP:
                conv1_chunk(2 * (q + 2), xts, ps1s)
            posts(5, c3_step(5))
            if q + 2 < NP:
                h1_chunk(2 * (q + 2), ps1s, h1s)
                conv1_chunk(2 * (q + 2) + 1, xts, ps1s)
            posts(6, c3_step(6))
            posts(7, c3_step(7))
            if last:
                nc.gpsimd.dma_start(dsts[0][:, 4:, :], o_ts[0][:, 4:, :])
                nc.gpsimd.dma_start(dsts[1][:, 4:, :], o_ts[1][:, 4:, :])
            if q + 2 < NP:
                h1_chunk(2 * (q + 2) + 1, ps1s, h1s)
            for ab in range(2):
                xts.pop(2 * (q + 2) + ab, None)

        for r in range(repeat):
            last = r == repeat - 1
            xts, ps1s, h1s, h2s = {}, {}, {}, {}
            for j in range(min(6, n)):
                xts[j] = emit_load(j, halves=2 if j == 0 else 1)
            for i in (0, 1):
                conv1_chunk(i, xts, ps1s)
                h1_chunk(i, ps1s, h1s)
            for i in (2, 3):
                conv1_chunk(i, xts, ps1s)
                h1_chunk(i, ps1s, h1s)
            alloc_h2(0, h2s)
            c2h2_m(0, 0, h1s, h2s)
            c2h2_m(0, 1, h1s, h2s)
            alloc_h2(1, h2s)
            for q in range(NP):
                emit_pair_iter(q, xts, ps1s, h1s, h2s, last)
    return nc


class _Exec:
    """Compile-once PJRT executor for the SPMD bass program (axon backend)."""

    def __init__(self, nc, n_cores):
        install_neuronx_cc_hook()
        self.n_cores = n_cores
        partition_name = nc.partition_id_tensor.name if nc.partition_id_tensor else None
        in_names, out_names, out_avals, zero_outs = [], [], [], []
        for alloc in nc.m.functions[0].allocations:
            if not isinstance(alloc, mybir.MemoryLocationSet):
                continue
            name = alloc.memorylocations[0].name
            if alloc.kind == "ExternalInput":
                if name != partition_name:
                    in_names.append(name)
            elif alloc.kind == "ExternalOutput":
                shape = tuple(alloc.tensor_shape)
                dtype = mybir.dt.np(alloc.dtype)
                out_names.append(name)
                out_avals.append(jax.core.ShapedArray(shape, dtype))
                zero_outs.append(np.zeros(shape, dtype))
        self.in_names, self.out_names, self.zero_outs = in_names, out_names, zero_outs
        n_params = len(in_names)
        all_in = list(in_names) + list(out_names)
        if partition_name is not None:
            all_in.append(partition_name)

        def _body(*args):
            operands = list(args)
            if partition_name is not None:
                operands.append(bass2jax.partition_id_tensor())
            return tuple(_bass_exec_p.bind(
                *operands,
                out_avals=tuple(out_avals),
                in_names=tuple(all_in),
                out_names=tuple(out_names),
                lowering_input_output_aliases=(),
                sim_require_finite=True,
                sim_require_nnan=True,
                nc=nc,
            ))

        devices = jax.devices()[:n_cores]
        assert len(devices) == n_cores, f"need {n_cores} cores, have {len(jax.devices())}"
        mesh = Mesh(np.asarray(devices), ("core",))
        specs = (PartitionSpec("core"),) * (n_params + len(out_names))
        self._fn = jax.jit(
            shard_map(_body, mesh=mesh, in_specs=specs,
                      out_specs=(PartitionSpec("core"),) * len(out_names),
                      check_rep=False),
            keep_unused=True,
        )

    def stage(self, in_maps):
        per_core = [[np.asarray(m[n]) for n in self.in_names] for m in in_maps]
        args = [np.concatenate([per_core[c][i] for c in range(self.n_cores)], axis=0)
                for i in range(len(self.in_names))]
        args += [np.zeros((self.n_cores * z.shape[0], *z.shape[1:]), z.dtype)
                 for z in self.zero_outs]
        return args

    def run_staged(self, args):
        out = self._fn(*args)
        jax.block_until_ready(out)
        return out

    def fetch(self, out_arrs):
        return [
            {n: np.asarray(out_arrs[i]).reshape(self.n_cores, *self.zero_outs[i].shape)[c]
             for i, n in enumerate(self.out_names)}
            for c in range(self.n_cores)
        ]


_EXEC_CACHE = {}


def _get_exec(repeat: int = 1):
    if repeat not in _EXEC_CACHE:
        nc = build_bass(repeat)
        removed = _dedup_ldweights(nc)
        assert removed > 0
        _split_excess_waits(nc)
        _EXEC_CACHE[repeat] = _Exec(nc, NCORES)
    return _EXEC_CACHE[repeat]


def _prepare_in_maps(x, gate_values, W1, b1, W2, b2, W3, b3):
    import ml_dtypes
    f8np = ml_dtypes.float8_e4m3

    x = np.asarray(x, dtype=np.float32)
    gate = np.asarray(gate_values, dtype=np.float32)
    W1 = np.asarray(W1, dtype=np.float32)
    W2 = np.asarray(W2, dtype=np.float32)
    W3 = np.asarray(W3, dtype=np.float32)
    b1 = np.asarray(b1, dtype=np.float32)
    b2 = np.asarray(b2, dtype=np.float32)
    b3 = np.asarray(b3, dtype=np.float32)

    def to_f8(a):
        return np.clip(a, -240.0, 240.0).astype(f8np)

    xs_f8 = to_f8(x.reshape(B, C, HW))
    xs_f8 = np.ascontiguousarray(
        xs_f8.reshape(B, KO, 128, NCHUNKS, NC_CHUNK).transpose(0, 3, 2, 1, 4)
    ).reshape(B, NCHUNKS, 128, KO * NC_CHUNK)
    w1t = np.ascontiguousarray(
        to_f8((S1 * W1).T.reshape(KO, 128, MID).transpose(1, 0, 2)))
    w2t = np.ascontiguousarray(
        to_f8((S2 * W2).T.reshape(M2, 128, MID).transpose(1, 0, 2)))
    w3t = np.ascontiguousarray(
        to_f8((S3 * W3).T.reshape(M2, 128, C).transpose(1, 0, 2)))
    b3v = np.ascontiguousarray((OUT_SCALE * b3).reshape(KO, 128).T.astype(np.float32))

    g_all = np.maximum(gate, 0.0)
    in_maps = []
    for c in range(NCORES):
        vecs = np.zeros((128, BPC, 8), np.float32)
        for bl in range(BPC):
            g = g_all[c * BPC + bl]
            a1s = g * (SH1 / S1)
            a1b = g * b1 * SH1
            a2s = g * (SH2 / (S2 * SH1))
            a2b = g * b2 * SH2
            for m in range(M2):
                sl = slice(m * 128, (m + 1) * 128)
                vecs[:, bl, m] = a1s[sl]
                vecs[:, bl, 2 + m] = a1b[sl]
                vecs[:, bl, 4 + m] = a2s[sl]
                vecs[:, bl, 6 + m] = a2b[sl]
        in_maps.append({
            "xs": xs_f8[c * BPC:(c + 1) * BPC],
            "w1t": w1t, "w2t": w2t, "w3t": w3t,
            "vecs": vecs, "b3v": b3v,
        })
    return in_maps


def kernel(x, gate_values, W1, b1, W2, b2, W3, b3):
    in_maps = _prepare_in_maps(x, gate_values, W1, b1, W2, b2, W3, b3)
    ex = _get_exec(int(os.environ.get("BOTTLENECK_REPEAT", "1")))
    args = ex.stage(in_maps)
    try:
        out_arrs = ex.run_staged(args)
    except Exception:
        time.sleep(2.0)
        out_arrs = ex.run_staged(args)
    outs = ex.fetch(out_arrs)
    t = np.concatenate([o["ys"] for o in outs], axis=0)
    relu3 = np.ascontiguousarray(
        t.reshape(B, NCHUNKS, 128, KO, NC_CHUNK).transpose(0, 3, 2, 1, 4)
    ).reshape(B, C, HW).astype(np.float32)
    y = np.asarray(x, dtype=np.float32).reshape(B, C, HW) + relu3 * (1.0 / OUT_SCALE)
    return y.reshape(B, C, 56, 56)



# revision 1
# speedup vs baseline: 1.0430x; 1.0430x over previous
"""TRN2 Bass kernel for nn_BottleneckA (gated bottleneck MLP over 1x1 convs).

Computation (reference):
    h1 = relu(g * (W1 @ x + b1))    g = relu(gate)   per (batch, mid-channel)
    h2 = relu(g * (W2 @ h1 + b2))
    y  = relu(W3 @ h2 + b3) + x

All three matmuls run in fp8e4m3 with perf_mode=DoubleRow (2 fp8 weights
per PE cell -> 256-deep contraction per pass, ~2x bf16 FLOP rate).
Weights are pre-scaled by powers of two (S1=32, S2=16, S3=16) and the
activations stored scaled (h1 x4, h2 x2) to sit in e4m3's normal range;
every scale folds into the per-(batch,channel) ACT gate scale/bias, so
no extra device ops. The device returns t = 32*relu(W3 h2 + b3) in fp8;
the host applies y = x + t/32 with the exact fp32 x it already holds
(final rel-rms error ~7.8e-3, dominated by fp8 quantization diluted by
the residual: relu3 rms is only ~0.087 of y rms).

Sharding: data-parallel over batch B=16 across 8 NeuronCores (2/core),
weights replicated. Per core, each batch's [C=1024, HW=3136] activation
is processed in 7 spatial chunks of 448 columns (one PSUM bank per
matmul output). x/y are chunk-tiled on the host so chunk DMAs move 3584
contiguous bytes per partition (sub-512B runs run at half DMA rate).

Spatial chunks are processed in PAIRS sharing weight loads: conv2/conv3
emit LDW, MM(chunkA), MM(chunkB) (singleton accumulation groups stay
adjacent through the Tile scheduler) and a post-legalization pass
deletes the duplicate back-to-back LDWs. LDWEIGHTS only partially
overlaps DoubleRow matmuls on this hardware (~90ns effective per LDW on
top of the 211ns MM), so fewer LDWs means real PE time. conv1's rigid
4-MM accumulation chains are scheduled contiguously per chunk and cannot
share LDWs. conv3's per-chunk output posts (relu + b3, PSUM->fp8) are
split 12-DVE / 4-ACT per pair, balancing both engines under the PE pace;
stores ride the otherwise-idle GPSIMD SWDGE queue so no compute queue
ever blocks on a store's data dependency.
"""
import os
import time

import numpy as np

import concourse.bass as bass
import concourse.tile as tile
from concourse import mybir, bass2jax
from concourse.bass2jax import _bass_exec_p, install_neuronx_cc_hook
from contextlib import ExitStack

import jax
from jax.sharding import Mesh, PartitionSpec
from jax.experimental.shard_map import shard_map

B, C, MID, HW = 16, 1024, 256, 56 * 56
NCORES = 8
BPC = B // NCORES
NC_CHUNK = 448
NCHUNKS = HW // NC_CHUNK
KO = C // 128
M2 = MID // 128
f32 = mybir.dt.float32
f8 = mybir.dt.float8e4

S1, S2, S3 = 32.0, 16.0, 16.0
SH1, SH2 = 4.0, 2.0
OUT_SCALE = S3 * SH2

_EVS_CAP = 2


def _split_excess_waits(nc):
    """This container's walrus accepts only 1 sync-wait slot on most ISA
    structs while Tile emits 2-3; hoist the excess onto preceding
    InstEventSemaphore ops on the same (FIFO) engine queue."""
    counter = [0]
    for fn in nc.m.functions:
        for blk in fn.blocks:
            new_insts = []
            for inst in blk.instructions:
                si = inst.sync_info
                waits = list(si.on_wait) if si is not None else []
                cap = _EVS_CAP if type(inst).__name__ == "InstEventSemaphore" else 1
                if len(waits) > cap:
                    excess, keep = waits[: len(waits) - cap], waits[len(waits) - cap:]
                    while excess:
                        chunk, excess = excess[:_EVS_CAP], excess[_EVS_CAP:]
                        counter[0] += 1
                        new_insts.append(mybir.InstEventSemaphore(
                            name=f"EVSW-{counter[0]}-{inst.name}",
                            engine=inst.engine,
                            ins=[], outs=[],
                            sync_info=mybir.SyncInfo(on_wait=list(chunk), on_update=[]),
                        ))
                    inst.sync_info = mybir.SyncInfo(
                        on_wait=keep, on_update=list(si.on_update))
                new_insts.append(inst)
            blk.instructions = new_insts


def _ldw_sig(inst):
    a = inst.ins[0]
    return (a.memref, a.offset, tuple(tuple(p) for p in a.ap), str(a.dtype),
            str(inst.perf_mode))


def _dedup_ldweights(nc):
    """Remove an InstLdweights whose weights AP matches the immediately
    preceding LDW on the PE stream with only Matmults in between (the
    weights are already loaded). Waits on the removed LDW migrate to the
    next instruction (hoisted later by _split_excess_waits)."""
    removed = 0
    PE = mybir.EngineType.PE
    for fn in nc.m.functions:
        for blk in fn.blocks:
            new = []
            last_sig = None
            pending_waits = []
            for inst in blk.instructions:
                tn = type(inst).__name__
                if getattr(inst, "engine", None) == PE:
                    if tn == "InstLdweights":
                        sig = _ldw_sig(inst)
                        if sig == last_sig:
                            si = inst.sync_info
                            if si is not None:
                                assert not si.on_update, "LDW with updates"
                                pending_waits.extend(si.on_wait)
                            removed += 1
                            continue
                        last_sig = sig
                    elif tn not in ("InstMatmult", "InstEventSemaphore"):
                        # control flow / drains: be conservative
                        last_sig = None
                    if pending_waits and tn != "InstLdweights":
                        si = inst.sync_info
                        waits = list(si.on_wait) if si is not None else []
                        upd = list(si.on_update) if si is not None else []
                        inst.sync_info = mybir.SyncInfo(
                            on_wait=pending_waits + waits, on_update=upd)
                        pending_waits = []
                new.append(inst)
            # leftover waits at block end: attach to nothing is unsafe
            assert not pending_waits
            blk.instructions = new
    return removed


def build_bass(repeat: int = 1) -> bass.Bass:
    nc = bass.Bass(trn_type="TRN2")
    xs = nc.dram_tensor("xs", [BPC, NCHUNKS, 128, KO * NC_CHUNK], f8,
                        kind="ExternalInput")
    w1t = nc.dram_tensor("w1t", [128, KO, MID], f8, kind="ExternalInput")
    w2t = nc.dram_tensor("w2t", [128, M2, MID], f8, kind="ExternalInput")
    w3t = nc.dram_tensor("w3t", [128, M2, C], f8, kind="ExternalInput")
    vecs = nc.dram_tensor("vecs", [128, BPC, 8], f32, kind="ExternalInput")
    b3v = nc.dram_tensor("b3v", [128, KO], f32, kind="ExternalInput")
    ys = nc.dram_tensor("ys", [BPC, NCHUNKS, 128, KO * NC_CHUNK], f8,
                        kind="ExternalOutput")

    Relu = mybir.ActivationFunctionType.Relu
    DR = mybir.MatmulPerfMode.DoubleRow

    with tile.TileContext(nc) as tc, ExitStack() as ctx:
        wpool = ctx.enter_context(tc.tile_pool(name="w", bufs=1))
        xpool = ctx.enter_context(tc.tile_pool(name="x", bufs=8))
        hpool = ctx.enter_context(tc.tile_pool(name="h", bufs=4))
        opool = ctx.enter_context(tc.tile_pool(name="o", bufs=4))
        pp1 = ctx.enter_context(tc.tile_pool(name="pp1", bufs=3, space="PSUM"))
        pp2 = ctx.enter_context(tc.tile_pool(name="pp2", bufs=2, space="PSUM"))
        pp3 = ctx.enter_context(tc.tile_pool(name="pp3", bufs=3, space="PSUM"))

        w1_sb = wpool.tile([128, KO, MID], f8, tag="w1")
        nc.scalar.dma_start(w1_sb[:], w1t[:])
        v_sb = wpool.tile([128, BPC, 8], f32, tag="v")
        nc.scalar.dma_start(v_sb[:], vecs[:])
        w2_sb = wpool.tile([128, M2, MID], f8, tag="w2")
        nc.scalar.dma_start(w2_sb[:], w2t[:])
        w3_sb = wpool.tile([128, M2, C], f8, tag="w3")
        nc.scalar.dma_start(w3_sb[:], w3t[:])
        b3_sb = wpool.tile([128, KO], f32, tag="b3")
        nc.scalar.dma_start(b3_sb[:], b3v[:])

        chunks = [(b, ci) for b in range(BPC) for ci in range(NCHUNKS)]
        n = len(chunks)
        NP = n // 2            # chunk pairs

        def emit_load(i, halves=1):
            b, ci = chunks[i]
            x_t = xpool.tile([128, KO, NC_CHUNK], f8, tag="xt")
            src = xs[b, ci].rearrange("p (ko n) -> p ko n", ko=KO)
            step = KO // halves
            for h in range(halves):
                nc.sync.dma_start(x_t[:, h * step:(h + 1) * step, :],
                                  src[:, h * step:(h + 1) * step, :])
            return x_t

        def conv1_chunk(i, xts, ps1s):
            """per-chunk conv1 (the scheduler keeps each accumulation
            chain contiguous, so chunk-pair LDW sharing is not possible
            here; chains stay per chunk)."""
            x_t = xts[i]
            ps1s[i] = []
            for m in range(M2):
                ps = pp1.tile([128, NC_CHUNK], f32, tag="ps1", name="ps1t")
                for kd in range(KO // 2):
                    nc.tensor.matmul(
                        ps[:], w1_sb[:, 2 * kd:2 * kd + 2,
                                     m * 128:(m + 1) * 128],
                        x_t[:, 2 * kd:2 * kd + 2, :],
                        start=(kd == 0), stop=(kd == KO // 2 - 1),
                        perf_mode=DR)
                ps1s[i].append(ps)

        def h1_chunk(i, ps1s, h1s):
            b, _ = chunks[i]
            h1 = hpool.tile([128, M2, NC_CHUNK], f8, tag="h1", name="h1t")
            for m in range(M2):
                nc.scalar.activation(h1[:, m, :], ps1s[i][m][:], Relu,
                                     bias=v_sb[:, b, 2 + m:3 + m],
                                     scale=v_sb[:, b, m:m + 1])
            h1s[i] = h1
            del ps1s[i]

        def c2h2_m(q, m, h1s, h2s):
            """conv2 m-tile for both chunks of pair q (one shared LDW:
            the two MMs are singleton groups and stay adjacent) + h2."""
            pss = []
            for ab in range(2):
                i = 2 * q + ab
                ps = pp2.tile([128, NC_CHUNK], f32, tag="ps2", name="ps2t")
                nc.tensor.matmul(ps[:], w2_sb[:, :, m * 128:(m + 1) * 128],
                                 h1s[i][:, :, :], start=True, stop=True,
                                 perf_mode=DR)
                pss.append(ps)
            for ab in range(2):
                i = 2 * q + ab
                b, _ = chunks[i]
                nc.scalar.activation(h2s[i][:, m, :], pss[ab][:], Relu,
                                     bias=v_sb[:, b, 6 + m:7 + m],
                                     scale=v_sb[:, b, 4 + m:5 + m])

        def alloc_h2(q, h2s):
            for ab in range(2):
                h2s[2 * q + ab] = hpool.tile([128, M2, NC_CHUNK], f8,
                                             tag="h2", name="h2t")

        def emit_pair_iter(q, xts, ps1s, h1s, h2s, last):
            """Iter q: conv3+posts+stores for pair q; conv2/h2 for pair
            q+1; conv1+h1 for pair q+2; loads for pair q+3. The PE stream
            interleaves conv3 pair-steps (shared LDW) with the rest."""
            if q + 3 < NP:
                for ab in range(2):
                    xts[2 * (q + 3) + ab] = emit_load(2 * (q + 3) + ab)
            h2A, h2B = h2s.pop(2 * q), h2s.pop(2 * q + 1)
            if q + 2 < NP:
                alloc_h2(q + 2, h2s)
            o_ts, dsts = [], []
            for ab in range(2):
                i = 2 * q + ab
                b, ci = chunks[i]
                o_ts.append(opool.tile([128, KO, NC_CHUNK], f8, tag="ot",
                                       name="ot"))
                dsts.append(ys[b, ci].rearrange("p (m n) -> p m n", m=KO))

            def c3_step(m8):
                """one shared-weight conv3 step: LDW w3[m8]; MM A; MM B."""
                pss = []
                for h2 in (h2A, h2B):
                    ps = pp3.tile([128, NC_CHUNK], f32, tag="ps3",
                                  name="ps3t")
                    nc.tensor.matmul(ps[:],
                                     w3_sb[:, :, m8 * 128:(m8 + 1) * 128],
                                     h2[:, :, :], start=True, stop=True,
                                     perf_mode=DR)
                    pss.append(ps)
                return pss

            def posts(m8, pss):
                on_act = m8 in (0, 2)
                for ab in range(2):
                    if on_act:
                        nc.scalar.activation(o_ts[ab][:, m8, :], pss[ab][:],
                                             Relu, bias=b3_sb[:, m8:m8 + 1],
                                             scale=1.0)
                    else:
                        nc.vector.tensor_scalar(o_ts[ab][:, m8, :],
                                                pss[ab][:],
                                                b3_sb[:, m8:m8 + 1], 0.0,
                                                mybir.AluOpType.add,
                                                mybir.AluOpType.max)

            posts(0, c3_step(0))
            posts(1, c3_step(1))
            if q + 1 < NP:
                c2h2_m(q + 1, 0, h1s, h2s)
            posts(2, c3_step(2))
            posts(3, c3_step(3))
            if last:
                nc.gpsimd.dma_start(dsts[0][:, :4, :], o_ts[0][:, :4, :])
                nc.gpsimd.dma_start(dsts[1][:, :4, :], o_ts[1][:, :4, :])
            if q + 1 < NP:
                c2h2_m(q + 1, 1, h1s, h2s)
            posts(4, c3_step(4))
            if q + 2 < NP:
                conv1_chunk(2 * (q + 2), xts, ps1s)
            posts(5, c3_step(5))
            if q + 2 < NP:
                h1_chunk(2 * (q + 2), ps1s, h1s)
                conv1_chunk(2 * (q + 2) + 1, xts, ps1s)
            posts(6, c3_step(6))
            posts(7, c3_step(7))
            if last:
                nc.gpsimd.dma_start(dsts[0][:, 4:, :], o_ts[0][:, 4:, :])
                nc.gpsimd.dma_start(dsts[1][:, 4:, :], o_ts[1][:, 4:, :])
            if q + 2 < NP:
                h1_chunk(2 * (q + 2) + 1, ps1s, h1s)
            for ab in range(2):
                xts.pop(2 * (q + 2) + ab, None)

        for r in range(repeat):
            last = r == repeat - 1
            xts, ps1s, h1s, h2s = {}, {}, {}, {}
            for j in range(min(6, n)):
                xts[j] = emit_load(j, halves=2 if j == 0 else 1)
            for i in (0, 1):
                conv1_chunk(i, xts, ps1s)
                h1_chunk(i, ps1s, h1s)
            for i in (2, 3):
                conv1_chunk(i, xts, ps1s)
                h1_chunk(i, ps1s, h1s)
            alloc_h2(0, h2s)
            c2h2_m(0, 0, h1s, h2s)
            c2h2_m(0, 1, h1s, h2s)
            alloc_h2(1, h2s)
            for q in range(NP):
                emit_pair_iter(q, xts, ps1s, h1s, h2s, last)
    return nc


class _Exec:
    """Compile-once PJRT executor for the SPMD bass program (axon backend)."""

    def __init__(self, nc, n_cores):
        install_neuronx_cc_hook()
        self.n_cores = n_cores
        partition_name = nc.partition_id_tensor.name if nc.partition_id_tensor else None
        in_names, out_names, out_avals, zero_outs = [], [], [], []
        for alloc in nc.m.functions[0].allocations:
            if not isinstance(alloc, mybir.MemoryLocationSet):
                continue
            name = alloc.memorylocations[0].name
            if alloc.kind == "ExternalInput":
                if name != partition_name:
                    in_names.append(name)
            elif alloc.kind == "ExternalOutput":
                shape = tuple(alloc.tensor_shape)
                dtype = mybir.dt.np(alloc.dtype)
                out_names.append(name)
                out_avals.append(jax.core.ShapedArray(shape, dtype))
                zero_outs.append(np.zeros(shape, dtype))
        self.in_names, self.out_names, self.zero_outs = in_names, out_names, zero_outs
        n_params = len(in_names)
        all_in = list(in_names) + list(out_names)
        if partition_name is not None:
            all_in.append(partition_name)

        def _body(*args):
            operands = list(args)
            if partition_name is not None:
                operands.append(bass2jax.partition_id_tensor())
            return tuple(_bass_exec_p.bind(
                *operands,
                out_avals=tuple(out_avals),
                in_names=tuple(all_in),
                out_names=tuple(out_names),
                lowering_input_output_aliases=(),
                sim_require_finite=True,
                sim_require_nnan=True,
                nc=nc,
            ))

        devices = jax.devices()[:n_cores]
        assert len(devices) == n_cores, f"need {n_cores} cores, have {len(jax.devices())}"
        mesh = Mesh(np.asarray(devices), ("core",))
        specs = (PartitionSpec("core"),) * (n_params + len(out_names))
        self._fn = jax.jit(
            shard_map(_body, mesh=mesh, in_specs=specs,
                      out_specs=(PartitionSpec("core"),) * len(out_names),
                      check_rep=False),
            keep_unused=True,
        )

    def stage(self, in_maps):
        per_core = [[np.asarray(m[n]) for n in self.in_names] for m in in_maps]
        args = [np.concatenate([per_core[c][i] for c in range(self.n_cores)], axis=0)
                for i in range(len(self.in_names))]
        args += [np.zeros((self.n_cores * z.shape[0], *z.shape[1:]), z.dtype)
                 for z in self.zero_outs]
        return args

    def run_staged(self, args):
        out = self._fn(*args)
        jax.block_until_ready(out)
        return out

    def fetch(self, out_arrs):
        return [
            {n: np.asarray(out_arrs[i]).reshape(self.n_cores, *self.zero_outs[i].shape)[c]
             for i, n in enumerate(self.out_names)}
            for c in range(self.n_cores)
        ]


_EXEC_CACHE = {}


def _get_exec(repeat: int = 1):
    if repeat not in _EXEC_CACHE:
        nc = build_bass(repeat)
        removed = _dedup_ldweights(nc)
        assert removed > 0
        _split_excess_waits(nc)
        _EXEC_CACHE[repeat] = _Exec(nc, NCORES)
    return _EXEC_CACHE[repeat]


def _prepare_in_maps(x, gate_values, W1, b1, W2, b2, W3, b3):
    import ml_dtypes
    f8np = ml_dtypes.float8_e4m3

    x = np.asarray(x, dtype=np.float32)
    gate = np.asarray(gate_values, dtype=np.float32)
    W1 = np.asarray(W1, dtype=np.float32)
    W2 = np.asarray(W2, dtype=np.float32)
    W3 = np.asarray(W3, dtype=np.float32)
    b1 = np.asarray(b1, dtype=np.float32)
    b2 = np.asarray(b2, dtype=np.float32)
    b3 = np.asarray(b3, dtype=np.float32)

    def to_f8(a):
        return np.clip(a, -240.0, 240.0).astype(f8np)

    xs_f8 = to_f8(x.reshape(B, C, HW))
    xs_f8 = np.ascontiguousarray(
        xs_f8.reshape(B, KO, 128, NCHUNKS, NC_CHUNK).transpose(0, 3, 2, 1, 4)
    ).reshape(B, NCHUNKS, 128, KO * NC_CHUNK)
    w1t = np.ascontiguousarray(
        to_f8((S1 * W1).T.reshape(KO, 128, MID).transpose(1, 0, 2)))
    w2t = np.ascontiguousarray(
        to_f8((S2 * W2).T.reshape(M2, 128, MID).transpose(1, 0, 2)))
    w3t = np.ascontiguousarray(
        to_f8((S3 * W3).T.reshape(M2, 128, C).transpose(1, 0, 2)))
    b3v = np.ascontiguousarray((OUT_SCALE * b3).reshape(KO, 128).T.astype(np.float32))

    g_all = np.maximum(gate, 0.0)
    in_maps = []
    for c in range(NCORES):
        vecs = np.zeros((128, BPC, 8), np.float32)
        for bl in range(BPC):
            g = g_all[c * BPC + bl]
            a1s = g * (SH1 / S1)
            a1b = g * b1 * SH1
            a2s = g * (SH2 / (S2 * SH1))
            a2b = g * b2 * SH2
            for m in range(M2):
                sl = slice(m * 128, (m + 1) * 128)
                vecs[:, bl, m] = a1s[sl]
                vecs[:, bl, 2 + m] = a1b[sl]
                vecs[:, bl, 4 + m] = a2s[sl]
                vecs[:, bl, 6 + m] = a2b[sl]
        in_maps.append({
            "xs": xs_f8[c * BPC:(c + 1) * BPC],
            "w1t": w1t, "w2t": w2t, "w3t": w3t,
            "vecs": vecs, "b3v": b3v,
        })
    return in_maps


def kernel(x, gate_values, W1, b1, W2, b2, W3, b3):
    in_maps = _prepare_in_maps(x, gate_values, W1, b1, W2, b2, W3, b3)
    ex = _get_exec(int(os.environ.get("BOTTLENECK_REPEAT", "1")))
    args = ex.stage(in_maps)
    try:
        out_arrs = ex.run_staged(args)
    except Exception:
        time.sleep(2.0)
        out_arrs = ex.run_staged(args)
    outs = ex.fetch(out_arrs)
    t = np.concatenate([o["ys"] for o in outs], axis=0)
    relu3 = np.ascontiguousarray(
        t.reshape(B, NCHUNKS, 128, KO, NC_CHUNK).transpose(0, 3, 2, 1, 4)
    ).reshape(B, C, HW).astype(np.float32)
    y = np.asarray(x, dtype=np.float32).reshape(B, C, HW) + relu3 * (1.0 / OUT_SCALE)
    return y.reshape(B, C, 56, 56)

